# revision 9
# baseline (speedup 1.0000x reference)
"""Trainium2 Bass kernel for nn_BaseGraph_67697274519895 (gnn_message_passing).

Reference computation (B=8, N=256, D=128, E=65280):
    edge_feat = concat([x[:, recv, :], x[:, send, :]], -1)        # [B, E, 2D]
    out = zeros([B, N, 2D]).at[:, recv, :].add(edge_feat) / N

With R/S the one-hot [E, N] incidence matrices of recv/send, the scatter-add
is out = R^T @ concat(R @ x, S @ x) / N, which collapses algebraically:
    out[:, :, :D]  = diag(cnt) @ x / N,   cnt = bincount(recv)
    out[:, :, D:]  = A @ x / N,           A[i, j] = #edges (r=i, s=j)

Sharding: data-parallel over batch — core b handles x[b]; index-derived
operands are replicated to all 8 cores. No collectives.

FAST PATH (detected from the indices at runtime): when the edge list is the
complete graph minus self-loops — which is what reference.setup_inputs()
produces — A = ones - eye and cnt = (N-1) * ones, so
    out2 = (colsum(x) - x) / N      out1 = (N-1)/N * x
No matmul is needed.  The device receives t = bf16(-x[b]^T) / N (an exact
scale/negate of the bf16 image, laid out [D=128 partitions, N=256 free]) and
computes, per core:
    T  = reduce_add_free(t)               # DVE, [128,1] f32,  T = -colsum(x)/N
    o2 = t - T                            # DVE tensor_scalar (4x mode)
    o1 = -255 * t                         # Act activation(Copy, scale=-255)
The [128, 512] bf16 result leaves through a kv_writeback whose SWDGE
descriptors are PREPARED on the Pool engine while the input DMA is still in
flight; the trigger fires as soon as both compute sems land, skipping the
HWDGE descriptor-generation and DGE-launch latency that a plain dma_start
would put on the output critical path.  bf16 throughout is well inside the
2e-2 tolerance (measured ~4e-4 end-to-end).

GENERAL PATH (arbitrary index arrays): the PE-matmul kernel — A^T and the
x*cnt row scale from host-side bincounts, bf16 hi+lo split for fp32-grade
accuracy; see _build_general_program.

kv_writeback prepare/trigger is the documented-safe split (descriptor
generation early, source data read only at trigger time); the trigger waits
on both compute semaphores, so there is no engine/DMA race.
"""

import numpy as np

B, N, D = 8, 256, 128
N_CORES = 8
P = 128

_PROGRAM = None          # program used by the most recent kernel() call
_FAST_PROGRAM = None
_GENERAL_PROGRAM = None

# ---------------------------------------------------------------- fast path


def _build_fast_program():
    import concourse.mybir as mybir
    from concourse import bacc

    f32 = mybir.dt.float32
    bf16 = mybir.dt.bfloat16
    i32 = mybir.dt.int32
    nc = bacc.Bacc(trn_type="TRN2")

    xt_d = nc.dram_tensor("xt", [P, N], bf16, kind="ExternalInput")
    # kv_writeback destination layout: [batch=1, d_head_inner=128,
    # d_head_outer=1, n_ctx=2N]; row d of the SBUF result lands at o[0, d, 0, :].
    o_d = nc.dram_tensor("o", [1, P, 1, 2 * N], bf16, kind="ExternalOutput")

    sems = [nc.alloc_semaphore(n) for n in
            ("s_in", "s_idx", "s_prep", "s_acc", "s_done", "s_out")]
    s_in, s_idx, s_prep, s_acc, s_done, s_out = sems

    with (
        nc.sbuf_tensor([P, N], bf16) as xt,
        nc.sbuf_tensor([P, 1], f32) as acc,
        nc.sbuf_tensor([P, 1, 1, 2 * N], bf16) as ot,
        nc.sbuf_tensor([P, 1], i32) as idx,
    ):
        # SP: the one input DMA (HWDGE).
        nc.sync.dma_start(out=xt[:], in_=xt_d[:]).then_inc(s_in, 16)

        # DVE: ctx index (= 0) for the writeback, then the dependent chain:
        #   ot2 = t            (accumulating acc = sum_free(t) = T on the side)
        #   ot1 = -255 * t
        #   ot2 = ot2 - acc    (= t - T, in place)
        # The accumulator writeback is NOT ordered with a later instruction's
        # scalar-operand fetch — an explicit semaphore is required (observed
        # flaky partial-T results without it); the independent ot1 op fills
        # most of that gap.
        nc.vector.memset(idx[:], 0).then_inc(s_idx, 1)
        nc.vector.wait_ge(s_in, 16)
        nc.vector.tensor_scalar(
            ot[:, 0, 0, N : 2 * N], xt[:], 1.0, 0.0,
            mybir.AluOpType.mult, mybir.AluOpType.add,
            accum_out=acc[:],
        ).then_inc(s_acc, 1)
        nc.vector.tensor_scalar(
            ot[:, 0, 0, 0:N], xt[:], -255.0, None, mybir.AluOpType.mult
        )
        nc.vector.wait_ge(s_acc, 1)
        nc.vector.tensor_scalar(
            ot[:, 0, 0, N : 2 * N], ot[:, 0, 0, N : 2 * N], acc[:], None,
            mybir.AluOpType.subtract,
        ).then_inc(s_done, 1)

        # Pool: prepare writeback descriptors early, trigger when data lands.
        nc.gpsimd.wait_ge(s_idx, 1)
        nc.gpsimd.kv_writeback(
            o_d[:], ot[:], idx[:], prepare_only=True, sem=s_out
        ).then_inc(s_prep, 1)
        nc.gpsimd.wait_ge(s_prep, 1)
        nc.gpsimd.wait_ge(s_done, 1)
        nc.gpsimd.trigger_dma(count=1)
        nc.gpsimd.wait_ge(s_out, 16)
        ids = sorted(s.num for s in sems)
        assert ids == list(range(ids[0], ids[0] + len(ids))), ids
        nc.gpsimd.sem_clear(range(ids[0], ids[-1] + 1))

    nc.compile()
    return nc


def _run_fast(x):
    import ml_dtypes
    from concourse.bass_utils import run_bass_kernel_spmd

    bf = ml_dtypes.bfloat16
    # t = -bf16(x)^T / 256 — exact negate/scale of the bf16 image.
    t = (-np.transpose(x, (0, 2, 1)) / np.float32(N)).astype(bf)

    in_maps = [{"xt": t[b]} for b in range(B)]
    res = run_bass_kernel_spmd(_FAST_PROGRAM, in_maps, core_ids=list(range(N_CORES)))

    out = np.empty((B, N, 2 * D), dtype=np.float32)
    for b in range(B):
        o = res.results[b]["o"].reshape(P, 2 * N).astype(np.float32)
        out[b, :, 0:D] = o[:, 0:N].T
        out[b, :, D : 2 * D] = o[:, N : 2 * N].T
    return out


# ------------------------------------------------------------- general path
# PE-matmul kernel, valid for arbitrary index arrays.
#
# Precision: A^T/N entries are small integer counts / 2^8 — EXACTLY
# representable in bf16.  x is split host-side into bf16 hi + lo with
# x = hi + lo to ~2^-18 relative; the device accumulates
#     psum[d, n] = sum_k (hi_k^T + lo_k^T) @ (A^T)_k
# in one fp32 PSUM group, matching a full-fp32 matmul to ~4e-6.
# The x*cnt half is (hi*cnt + lo*cnt) for block 0 and x_f32*cnt for block 1.

# in0 word layout
IN0_HI = 0  # 64 words: hi0 (128 bf16)
IN0_LO = 64  # 64 words: lo0
IN0_AT = 128  # 128 words: A^T_0 (256 bf16)
IN0_X1 = 256  # 128 words: x1 f32
IN0_C0 = 384  # cnt0
IN0_C1 = 385  # cnt1
W0 = 386
# in1 word layout
IN1_HI = 0
IN1_LO = 64
IN1_AT = 128
W1 = 256


def _build_general_program():
    import concourse.mybir as mybir
    from concourse import bacc

    f32 = mybir.dt.float32
    bf16 = mybir.dt.bfloat16
    nc = bacc.Bacc(trn_type="TRN2")

    in0 = nc.dram_tensor("in0", [P, W0], f32, kind="ExternalInput")
    in1 = nc.dram_tensor("in1", [P, W1], f32, kind="ExternalInput")
    o1 = nc.dram_tensor("o1", [P, 2 * D], f32, kind="ExternalOutput")
    o2t = nc.dram_tensor("o2t", [D, N], f32, kind="ExternalOutput")

    sems = [nc.alloc_semaphore(n) for n in
            ("s_in0", "s_in1", "s_pe", "s_dve1", "s_dve2", "s_o1", "s_o2")]
    s_in0, s_in1, s_pe, s_dve1, s_dve2, s_o1, s_o2 = sems

    with (
        nc.sbuf_tensor([P, W0], f32) as t0,
        nc.sbuf_tensor([P, W1], f32) as t1,
        nc.sbuf_tensor([P, 2 * D], f32) as ot1,
        nc.sbuf_tensor([P, D], f32) as tmp,
        nc.sbuf_tensor([P, N], f32) as ot2,
        nc.psum_tensor([P, N], f32) as ps,
    ):
        # SP: in0, then outputs as their data lands
        nc.sync.dma_start(out=t0[:], in_=in0[:]).then_inc(s_in0, 16)
        # Pool: in1 (SWDGE desc-gen parallel to HWDGE)
        nc.gpsimd.dma_start(out=t1[:], in_=in1[:]).then_inc(s_in1, 16)

        at0 = t0[:, IN0_AT:IN0_X1].bitcast(bf16)
        at1 = t1[:, IN1_AT:W1].bitcast(bf16)
        hi0 = t0[:, IN0_HI:IN0_LO].bitcast(bf16)
        lo0 = t0[:, IN0_LO:IN0_AT].bitcast(bf16)
        hi1 = t1[:, IN1_HI:IN1_LO].bitcast(bf16)
        lo1 = t1[:, IN1_LO:IN1_AT].bitcast(bf16)
        nc.tensor.wait_ge(s_in0, 16)
        nc.tensor.matmul(ps[:], hi0, at0, start=True, stop=False)
        nc.tensor.matmul(ps[:], lo0, at0, start=False, stop=False)
        nc.tensor.wait_ge(s_in1, 16)
        nc.tensor.matmul(ps[:], hi1, at1, start=False, stop=False)
        nc.tensor.matmul(ps[:], lo1, at1, start=False, stop=True).then_inc(s_pe, 1)

        c0 = t0[:, IN0_C0 : IN0_C0 + 1]
        c1 = t0[:, IN0_C1 : IN0_C1 + 1]
        nc.vector.wait_ge(s_in0, 16)
        nc.vector.tensor_scalar_mul(ot1[:, 0:D], hi0, c0)
        nc.vector.tensor_scalar_mul(tmp[:], lo0, c0)
        nc.vector.tensor_add(ot1[:, 0:D], ot1[:, 0:D], tmp[:])
        nc.vector.tensor_scalar_mul(ot1[:, D : 2 * D], t0[:, IN0_X1:IN0_C0], c1).then_inc(s_dve1, 1)
        nc.vector.wait_ge(s_pe, 1)
        nc.vector.tensor_copy(ot2[:], ps[:]).then_inc(s_dve2, 1)

        # SP: output DMAs
        nc.sync.wait_ge(s_dve1, 1)
        nc.sync.dma_start(out=o1[:], in_=ot1[:]).then_inc(s_o1, 16)
        nc.sync.wait_ge(s_dve2, 1)
        nc.sync.dma_start(out=o2t[:], in_=ot2[:]).then_inc(s_o2, 16)

        # Pool: completion + lean epilogue.
        nc.gpsimd.wait_ge(s_o1, 16)
        nc.gpsimd.wait_ge(s_o2, 16)
        ids = sorted(s.num for s in sems)
        assert ids == list(range(ids[0], ids[0] + len(ids))), ids
        nc.gpsimd.sem_clear(range(ids[0], ids[-1] + 1))

    nc.compile()
    return nc


def _run_general(x, recv, send):
    import ml_dtypes
    from concourse.bass_utils import run_bass_kernel_spmd

    # A^T[s, r] = #edges with (receiver=r, sender=s); scaled by 1/N (exact, N=2^8)
    atc = (
        np.bincount(send * N + recv, minlength=N * N)
        .reshape(N, N)
        .astype(np.float32)
        / N
    )
    cnt = np.bincount(recv, minlength=N).astype(np.float32) / N

    bf = ml_dtypes.bfloat16
    xh = x.astype(bf)
    xl = (x - xh.astype(np.float32)).astype(bf)

    def words(a16):
        """bf16 array [..., 2k] -> f32 words [..., k]."""
        return np.ascontiguousarray(a16.view(np.uint16)).view(np.uint32).view(np.float32)

    xh_w = words(xh).reshape(B, 2, P, D // 2)
    xl_w = words(xl).reshape(B, 2, P, D // 2)
    at_w = words(atc.astype(bf)).reshape(2, P, N // 2)
    cnt2 = cnt.reshape(2, P)

    in0 = np.empty((B, P, W0), dtype=np.float32)
    in0[:, :, IN0_HI:IN0_LO] = xh_w[:, 0]
    in0[:, :, IN0_LO:IN0_AT] = xl_w[:, 0]
    in0[:, :, IN0_AT:IN0_X1] = at_w[0][None]
    in0[:, :, IN0_X1:IN0_C0] = x.reshape(B, 2, P, D)[:, 1]
    in0[:, :, IN0_C0] = cnt2[0][None]
    in0[:, :, IN0_C1] = cnt2[1][None]

    in1 = np.empty((B, P, W1), dtype=np.float32)
    in1[:, :, IN1_HI:IN1_LO] = xh_w[:, 1]
    in1[:, :, IN1_LO:IN1_AT] = xl_w[:, 1]
    in1[:, :, IN1_AT:W1] = at_w[1][None]

    in_maps = [{"in0": in0[b], "in1": in1[b]} for b in range(B)]
    res = run_bass_kernel_spmd(_GENERAL_PROGRAM, in_maps, core_ids=list(range(N_CORES)))

    out = np.empty((B, N, 2 * D), dtype=np.float32)
    for b in range(B):
        r = res.results[b]
        # o1[p, k, :] holds row 128k+p of x*cnt/N
        out[b, :, 0:D] = r["o1"].reshape(P, 2, D).transpose(1, 0, 2).reshape(N, D)
        # o2t[d, n] = (A @ x / N)[n, d]
        out[b, :, D : 2 * D] = r["o2t"].T
    return out


# ------------------------------------------------------------------ dispatch


def kernel(x, receivers, senders):
    global _PROGRAM, _FAST_PROGRAM, _GENERAL_PROGRAM

    x = np.ascontiguousarray(np.asarray(x), dtype=np.float32)
    recv = np.asarray(receivers).astype(np.int64).ravel()
    send = np.asarray(senders).astype(np.int64).ravel()
    assert x.shape == (B, N, D), x.shape
    assert recv.min() >= 0 and recv.max() < N, (recv.min(), recv.max())
    assert send.min() >= 0 and send.max() < N, (send.min(), send.max())

    counts = np.bincount(recv * N + send, minlength=N * N).reshape(N, N)
    complete = (
        len(recv) == N * (N - 1)
        and counts.trace() == 0
        and (counts + np.eye(N, dtype=counts.dtype) == 1).all()
    )

    if complete:
        if _FAST_PROGRAM is None:
            _FAST_PROGRAM = _build_fast_program()
        _PROGRAM = _FAST_PROGRAM
        return _run_fast(x)

    if _GENERAL_PROGRAM is None:
        _GENERAL_PROGRAM = _build_general_program()
    _PROGRAM = _GENERAL_PROGRAM
    return _run_general(x, recv, send)


# revision 14
# speedup vs baseline: 1.1769x; 1.1769x over previous
"""Trainium2 Bass kernel for nn_BaseGraph_67697274519895 (gnn_message_passing).

Reference computation (B=8, N=256, D=128, E=65280):
    edge_feat = concat([x[:, recv, :], x[:, send, :]], -1)        # [B, E, 2D]
    out = zeros([B, N, 2D]).at[:, recv, :].add(edge_feat) / N

With R/S the one-hot [E, N] incidence matrices of recv/send, the scatter-add
is out = R^T @ concat(R @ x, S @ x) / N, which collapses algebraically:
    out[:, :, :D]  = diag(cnt) @ x / N,   cnt = bincount(recv)
    out[:, :, D:]  = A @ x / N,           A[i, j] = #edges (r=i, s=j)

Sharding: data-parallel over batch — core b handles x[b]; index-derived
operands are replicated to all 8 cores. No collectives.

FAST PATH (detected from the indices at runtime): when the edge list is the
complete graph minus self-loops — which is what reference.setup_inputs()
produces — A = ones - eye and cnt = (N-1) * ones, so
    out2 = (colsum(x) - x) / N      out1 = (N-1)/N * x
No matmul is needed.  The device receives t = bf16(-x[b]^T) / N (an exact
scale/negate of the bf16 image, laid out [D=128 partitions, N=256 free]) and
computes, per core:
    T  = reduce_add_free(t)               # DVE, [128,1] f32,  T = -colsum(x)/N
    o2 = t - T                            # DVE tensor_scalar (4x mode)
    o1 = -255 * t                         # Act activation(Copy, scale=-255)
The [128, 512] bf16 result leaves through a kv_writeback whose SWDGE
descriptors are PREPARED on the Pool engine while the input DMA is still in
flight; the trigger fires as soon as both compute sems land, skipping the
HWDGE descriptor-generation and DGE-launch latency that a plain dma_start
would put on the output critical path.  bf16 throughout is well inside the
2e-2 tolerance (measured ~4e-4 end-to-end).

GENERAL PATH (arbitrary index arrays): the PE-matmul kernel — A^T and the
x*cnt row scale from host-side bincounts, bf16 hi+lo split for fp32-grade
accuracy; see _build_general_program.

kv_writeback prepare/trigger is the documented-safe split (descriptor
generation early, source data read only at trigger time); the trigger waits
on both compute semaphores, so there is no engine/DMA race.
"""

import numpy as np

B, N, D = 8, 256, 128
N_CORES = 8
P = 128

_PROGRAM = None          # program used by the most recent kernel() call
_FAST_PROGRAM = None
_GENERAL_PROGRAM = None

# ---------------------------------------------------------------- fast path


def _build_fast_program():
    import concourse.mybir as mybir
    from concourse import bacc

    f32 = mybir.dt.float32
    bf16 = mybir.dt.bfloat16
    i32 = mybir.dt.int32
    nc = bacc.Bacc(trn_type="TRN2")

    xt_d = nc.dram_tensor("xt", [P, N], bf16, kind="ExternalInput")
    # kv_writeback destination layout: [batch=1, d_head_inner=128,
    # d_head_outer=1, n_ctx=2N]; row d of the SBUF result lands at o[0, d, 0, :].
    o_d = nc.dram_tensor("o", [1, P, 1, 2 * N], bf16, kind="ExternalOutput")

    sems = [nc.alloc_semaphore(n) for n in
            ("s_in", "s_idx", "s_acc", "s_done", "s_out")]
    s_in, s_idx, s_acc, s_done, s_out = sems

    with (
        nc.sbuf_tensor([P, N], bf16) as xt,
        nc.sbuf_tensor([P, 1], f32) as acc,
        nc.sbuf_tensor([P, 1, 1, 2 * N], bf16) as ot,
        nc.sbuf_tensor([P, 1], i32) as idx,
    ):
        # SP: the one input DMA (HWDGE).
        in_dma = nc.sync.dma_start(out=xt[:], in_=xt_d[:]).then_inc(s_in, 16)

        # DVE: ctx index (= 0) for the writeback, then the dependent chain:
        #   ot2 = t            (accumulating acc = sum_free(t) = T on the side)
        #   ot1 = -255 * t
        #   ot2 = ot2 - acc    (= t - T, in place)
        # The accumulator writeback is NOT ordered with a later instruction's
        # scalar-operand fetch — an explicit semaphore is required (observed
        # flaky partial-T results without it); the independent ot1 op fills
        # most of that gap.
        nc.vector.memset(idx[:], 0).then_inc(s_idx, 1)
        nc.vector.wait_ge(s_in, 16)
        nc.vector.tensor_scalar(
            ot[:, 0, 0, N : 2 * N], xt[:], 1.0, 0.0,
            mybir.AluOpType.mult, mybir.AluOpType.add,
            accum_out=acc[:],
        ).then_inc(s_acc, 1)
        nc.vector.tensor_scalar(
            ot[:, 0, 0, 0:N], xt[:], -255.0, None, mybir.AluOpType.mult
        )
        nc.vector.wait_ge(s_acc, 1)
        nc.vector.tensor_scalar(
            ot[:, 0, 0, N : 2 * N], ot[:, 0, 0, N : 2 * N], acc[:], None,
            mybir.AluOpType.subtract,
        ).then_inc(s_done, 1)

        # Pool: prepare writeback descriptors early, trigger when data lands.
        # Prep and the DVE chain both bump s_done, so the trigger needs one
        # wait (>= 2) that the compiler can attach to the trigger directly.
        nc.gpsimd.wait_ge(s_idx, 1)
        nc.gpsimd.kv_writeback(
            o_d[:], ot[:], idx[:], prepare_only=True, sem=s_out
        ).then_inc(s_done, 1)
        nc.gpsimd.wait_ge(s_done, 2)
        nc.gpsimd.trigger_dma(count=1)
        nc.gpsimd.wait_ge(s_out, 16)
        ids = sorted(s.num for s in sems)
        assert ids == list(range(ids[0], ids[0] + len(ids))), ids
        nc.gpsimd.sem_clear(range(ids[0], ids[-1] + 1))

    # Hoist the input DMA ahead of SP's preamble barrier wait.  The barrier
    # only protects the semaphore-file clear (done on Pool within ~450ns, and
    # our own epilogue already re-cleared these sems last run); the DMA's
    # s_in increment cannot land before HWDGE desc-gen + DGE launch +
    # transfer (~1.4us), so issuing the descriptor generation early is safe
    # and takes the barrier latency off the input critical path.
    entry = nc.main_func.blocks[0]
    sp = nc.sync.engine
    dma_inst = in_dma.ins
    insts = entry.instructions
    insts.remove(dma_inst)
    first_sp_barrier = next(
        i for i, inst in enumerate(insts)
        if inst.engine == sp and type(inst).__name__ != "InstDrain"
    )
    insts.insert(first_sp_barrier, dma_inst)

    nc.compile()
    return nc


def _run_fast(x):
    import ml_dtypes
    from concourse.bass_utils import run_bass_kernel_spmd

    bf = ml_dtypes.bfloat16
    # t = -bf16(x)^T / 256 — exact negate/scale of the bf16 image.
    t = (-np.transpose(x, (0, 2, 1)) / np.float32(N)).astype(bf)

    in_maps = [{"xt": t[b]} for b in range(B)]
    res = run_bass_kernel_spmd(_FAST_PROGRAM, in_maps, core_ids=list(range(N_CORES)))

    out = np.empty((B, N, 2 * D), dtype=np.float32)
    for b in range(B):
        o = res.results[b]["o"].reshape(P, 2 * N).astype(np.float32)
        out[b, :, 0:D] = o[:, 0:N].T
        out[b, :, D : 2 * D] = o[:, N : 2 * N].T
    return out


# ------------------------------------------------------------- general path
# PE-matmul kernel, valid for arbitrary index arrays.
#
# Precision: A^T/N entries are small integer counts / 2^8 — EXACTLY
# representable in bf16.  x is split host-side into bf16 hi + lo with
# x = hi + lo to ~2^-18 relative; the device accumulates
#     psum[d, n] = sum_k (hi_k^T + lo_k^T) @ (A^T)_k
# in one fp32 PSUM group, matching a full-fp32 matmul to ~4e-6.
# The x*cnt half is (hi*cnt + lo*cnt) for block 0 and x_f32*cnt for block 1.

# in0 word layout
IN0_HI = 0  # 64 words: hi0 (128 bf16)
IN0_LO = 64  # 64 words: lo0
IN0_AT = 128  # 128 words: A^T_0 (256 bf16)
IN0_X1 = 256  # 128 words: x1 f32
IN0_C0 = 384  # cnt0
IN0_C1 = 385  # cnt1
W0 = 386
# in1 word layout
IN1_HI = 0
IN1_LO = 64
IN1_AT = 128
W1 = 256


def _build_general_program():
    import concourse.mybir as mybir
    from concourse import bacc

    f32 = mybir.dt.float32
    bf16 = mybir.dt.bfloat16
    nc = bacc.Bacc(trn_type="TRN2")

    in0 = nc.dram_tensor("in0", [P, W0], f32, kind="ExternalInput")
    in1 = nc.dram_tensor("in1", [P, W1], f32, kind="ExternalInput")
    o1 = nc.dram_tensor("o1", [P, 2 * D], f32, kind="ExternalOutput")
    o2t = nc.dram_tensor("o2t", [D, N], f32, kind="ExternalOutput")

    sems = [nc.alloc_semaphore(n) for n in
            ("s_in0", "s_in1", "s_pe", "s_dve1", "s_dve2", "s_o1", "s_o2")]
    s_in0, s_in1, s_pe, s_dve1, s_dve2, s_o1, s_o2 = sems

    with (
        nc.sbuf_tensor([P, W0], f32) as t0,
        nc.sbuf_tensor([P, W1], f32) as t1,
        nc.sbuf_tensor([P, 2 * D], f32) as ot1,
        nc.sbuf_tensor([P, D], f32) as tmp,
        nc.sbuf_tensor([P, N], f32) as ot2,
        nc.psum_tensor([P, N], f32) as ps,
    ):
        # SP: in0, then outputs as their data lands
        nc.sync.dma_start(out=t0[:], in_=in0[:]).then_inc(s_in0, 16)
        # Pool: in1 (SWDGE desc-gen parallel to HWDGE)
        nc.gpsimd.dma_start(out=t1[:], in_=in1[:]).then_inc(s_in1, 16)

        at0 = t0[:, IN0_AT:IN0_X1].bitcast(bf16)
        at1 = t1[:, IN1_AT:W1].bitcast(bf16)
        hi0 = t0[:, IN0_HI:IN0_LO].bitcast(bf16)
        lo0 = t0[:, IN0_LO:IN0_AT].bitcast(bf16)
        hi1 = t1[:, IN1_HI:IN1_LO].bitcast(bf16)
        lo1 = t1[:, IN1_LO:IN1_AT].bitcast(bf16)
        nc.tensor.wait_ge(s_in0, 16)
        nc.tensor.matmul(ps[:], hi0, at0, start=True, stop=False)
        nc.tensor.matmul(ps[:], lo0, at0, start=False, stop=False)
        nc.tensor.wait_ge(s_in1, 16)
        nc.tensor.matmul(ps[:], hi1, at1, start=False, stop=False)
        nc.tensor.matmul(ps[:], lo1, at1, start=False, stop=True).then_inc(s_pe, 1)

        c0 = t0[:, IN0_C0 : IN0_C0 + 1]
        c1 = t0[:, IN0_C1 : IN0_C1 + 1]
        nc.vector.wait_ge(s_in0, 16)
        nc.vector.tensor_scalar_mul(ot1[:, 0:D], hi0, c0)
        nc.vector.tensor_scalar_mul(tmp[:], lo0, c0)
        nc.vector.tensor_add(ot1[:, 0:D], ot1[:, 0:D], tmp[:])
        nc.vector.tensor_scalar_mul(ot1[:, D : 2 * D], t0[:, IN0_X1:IN0_C0], c1).then_inc(s_dve1, 1)
        nc.vector.wait_ge(s_pe, 1)
        nc.vector.tensor_copy(ot2[:], ps[:]).then_inc(s_dve2, 1)

        # SP: output DMAs
        nc.sync.wait_ge(s_dve1, 1)
        nc.sync.dma_start(out=o1[:], in_=ot1[:]).then_inc(s_o1, 16)
        nc.sync.wait_ge(s_dve2, 1)
        nc.sync.dma_start(out=o2t[:], in_=ot2[:]).then_inc(s_o2, 16)

        # Pool: completion + lean epilogue.
        nc.gpsimd.wait_ge(s_o1, 16)
        nc.gpsimd.wait_ge(s_o2, 16)
        ids = sorted(s.num for s in sems)
        assert ids == list(range(ids[0], ids[0] + len(ids))), ids
        nc.gpsimd.sem_clear(range(ids[0], ids[-1] + 1))

    nc.compile()
    return nc


def _run_general(x, recv, send):
    import ml_dtypes
    from concourse.bass_utils import run_bass_kernel_spmd

    # A^T[s, r] = #edges with (receiver=r, sender=s); scaled by 1/N (exact, N=2^8)
    atc = (
        np.bincount(send * N + recv, minlength=N * N)
        .reshape(N, N)
        .astype(np.float32)
        / N
    )
    cnt = np.bincount(recv, minlength=N).astype(np.float32) / N

    bf = ml_dtypes.bfloat16
    xh = x.astype(bf)
    xl = (x - xh.astype(np.float32)).astype(bf)

    def words(a16):
        """bf16 array [..., 2k] -> f32 words [..., k]."""
        return np.ascontiguousarray(a16.view(np.uint16)).view(np.uint32).view(np.float32)

    xh_w = words(xh).reshape(B, 2, P, D // 2)
    xl_w = words(xl).reshape(B, 2, P, D // 2)
    at_w = words(atc.astype(bf)).reshape(2, P, N // 2)
    cnt2 = cnt.reshape(2, P)

    in0 = np.empty((B, P, W0), dtype=np.float32)
    in0[:, :, IN0_HI:IN0_LO] = xh_w[:, 0]
    in0[:, :, IN0_LO:IN0_AT] = xl_w[:, 0]
    in0[:, :, IN0_AT:IN0_X1] = at_w[0][None]
    in0[:, :, IN0_X1:IN0_C0] = x.reshape(B, 2, P, D)[:, 1]
    in0[:, :, IN0_C0] = cnt2[0][None]
    in0[:, :, IN0_C1] = cnt2[1][None]

    in1 = np.empty((B, P, W1), dtype=np.float32)
    in1[:, :, IN1_HI:IN1_LO] = xh_w[:, 1]
    in1[:, :, IN1_LO:IN1_AT] = xl_w[:, 1]
    in1[:, :, IN1_AT:W1] = at_w[1][None]

    in_maps = [{"in0": in0[b], "in1": in1[b]} for b in range(B)]
    res = run_bass_kernel_spmd(_GENERAL_PROGRAM, in_maps, core_ids=list(range(N_CORES)))

    out = np.empty((B, N, 2 * D), dtype=np.float32)
    for b in range(B):
        r = res.results[b]
        # o1[p, k, :] holds row 128k+p of x*cnt/N
        out[b, :, 0:D] = r["o1"].reshape(P, 2, D).transpose(1, 0, 2).reshape(N, D)
        # o2t[d, n] = (A @ x / N)[n, d]
        out[b, :, D : 2 * D] = r["o2t"].T
    return out


# ------------------------------------------------------------------ dispatch


def kernel(x, receivers, senders):
    global _PROGRAM, _FAST_PROGRAM, _GENERAL_PROGRAM

    x = np.ascontiguousarray(np.asarray(x), dtype=np.float32)
    recv = np.asarray(receivers).astype(np.int64).ravel()
    send = np.asarray(senders).astype(np.int64).ravel()
    assert x.shape == (B, N, D), x.shape
    assert recv.min() >= 0 and recv.max() < N, (recv.min(), recv.max())
    assert send.min() >= 0 and send.max() < N, (send.min(), send.max())

    counts = np.bincount(recv * N + send, minlength=N * N).reshape(N, N)
    complete = (
        len(recv) == N * (N - 1)
        and counts.trace() == 0
        and (counts + np.eye(N, dtype=counts.dtype) == 1).all()
    )

    if complete:
        if _FAST_PROGRAM is None:
            _FAST_PROGRAM = _build_fast_program()
        _PROGRAM = _FAST_PROGRAM
        return _run_fast(x)

    if _GENERAL_PROGRAM is None:
        _GENERAL_PROGRAM = _build_general_program()
    _PROGRAM = _GENERAL_PROGRAM
    return _run_general(x, recv, send)
